# revision 3
# baseline (speedup 1.0000x reference)
"""Cox partial-likelihood NegativeLogLikelihood loss on 8 Trainium2 cores.

reference:
    mask[i, j] = (y[j] <= y[i])                       # (N, N)
    num[j] = sum_i exp(r_i) * mask[i, j]
    den[j] = sum_i mask[i, j]
    loss = -sum_j e_j * (r_j - log(num_j / den_j)) / sum_j e_j + 0.01 * ||W||_F

Strategy: shard columns j across the 8 cores (each core owns 2048 columns and
reads the full y / risk_pred, which are tiny).  Per core, the N x 2048 mask is
generated on-chip in [128, 2048] tiles (DVE tensor_scalar is_le, with a share
of tiles on ACT via the double-Sign identity sign(sign(y_i - y_j) + 1)) and
immediately contracted on the TensorEngine against lhsT = [exp_hi, exp_lo, 1]
(bf16, exp = hi + lo Dekker split for f32-grade accuracy) accumulating into
PSUM.  Each core reduces its own columns to a single partial scalar
out_c = -t_c / e_sum + 0.01 * ||W||_F / 8  (e_sum, ||W|| computed redundantly
from the replicated inputs), so the host-side unshard is a pure 8-way sum.
"""
import math

import numpy as np
import orjson

import concourse.bass as bass
import concourse.tile as tile
import concourse.mybir as mybir
from concourse.bass_utils import run_bass_kernel_spmd

F32 = mybir.dt.float32
BF16 = mybir.dt.bfloat16
I32 = mybir.dt.int32

N = 16384
NCORES = 8
JSHARD = N // NCORES            # 2048 columns per core
NT = N // 128                   # 128 i-tiles of 128 rows
NJJ = JSHARD // 512             # 4 matmul column groups per core
ACT_EVERY = 5                   # every 5th mask tile is produced on ScalarE

# ---------------------------------------------------------------------------
# Workaround for the installed walrus accepting at most ONE sync-wait command
# per TPB instruction: split multi-wait instructions into preceding
# single-wait EventSemaphore instructions on the same engine.
# ---------------------------------------------------------------------------

def _fix_bir_multiwait(bir_json: bytes) -> bytes:
    d = orjson.loads(bir_json)
    counter = 0
    for fn in d.get("functions", []):
        stack = list(fn.get("blocks", []))
        while stack:
            block = stack.pop()
            stack.extend(block.get("blocks", []))
            new_insts = []
            for inst in block.get("instructions", []):
                sync = inst.get("sync_info") or {}
                waits = sync.get("on_wait") or []
                if len(waits) > 1:
                    for w in waits[:-1]:
                        counter += 1
                        new_insts.append({
                            "debug": inst.get("debug", 0),
                            "engine": inst.get("engine"),
                            "ins": [],
                            "name": f"esw_fix_{counter}",
                            "opcode": "EventSemaphore",
                            "outs": [],
                            "sync_info": {"on_update": [], "on_wait": [w]},
                        })
                    sync["on_wait"] = [waits[-1]]
                new_insts.append(inst)
            block["instructions"] = new_insts
    return orjson.dumps(d)


_patched = False


def _install_bir_fix():
    global _patched
    if _patched:
        return
    _patched = True
    import concourse.bass_utils as bu
    import concourse.bass2jax as b2j

    orig = bu.compile_bir_kernel

    def patched(bir_json, tmpdir, neff_name="file.neff"):
        if isinstance(bir_json, str):
            bir_json = bir_json.encode()
        return orig(_fix_bir_multiwait(bir_json), tmpdir, neff_name)

    bu.compile_bir_kernel = patched
    b2j.compile_bir_kernel = patched


# ---------------------------------------------------------------------------
# Kernel build
# ---------------------------------------------------------------------------

def build_kernel() -> bass.Bass:
    nc = bass.Bass()
    Sign = mybir.ActivationFunctionType.Sign

    y_col = nc.dram_tensor("y_col", [128, NT], F32, kind="ExternalInput")
    r_col = nc.dram_tensor("r_col", [128, NT], F32, kind="ExternalInput")
    y_row = nc.dram_tensor("y_row", [1, JSHARD], F32, kind="ExternalInput")
    r_pf = nc.dram_tensor("r_pf", [128, NJJ * 4], F32, kind="ExternalInput")
    e_pf = nc.dram_tensor("e_pf", [128, NJJ * 4], I32, kind="ExternalInput")
    e_all = nc.dram_tensor("e_all", [128, NT], I32, kind="ExternalInput")
    w_t = nc.dram_tensor("w_t", [128, 1024], F32, kind="ExternalInput")
    out = nc.dram_tensor("out", [1, 1], F32, kind="ExternalOutput")

    with tile.TileContext(nc) as tc:
        with (
            tc.tile_pool(name="const", bufs=1) as const,
            tc.tile_pool(name="masks", bufs=6) as masks,
            tc.tile_pool(name="acttmp", bufs=2) as acttmp,
            tc.tile_pool(name="rows", bufs=2) as rows,
            tc.tile_pool(name="psacc", bufs=4, space="PSUM") as psacc,
            tc.tile_pool(name="pssum", bufs=1, space="PSUM") as pssum,
        ):
            # ---- input loads
            yb = const.tile([128, JSHARD], F32)
            nc.sync.dma_start(out=yb, in_=y_row.ap().to_broadcast([128, JSHARD]))
            ycol_sb = const.tile([128, NT], F32)
            nc.sync.dma_start(out=ycol_sb, in_=y_col[:, :])
            rcol_sb = const.tile([128, NT], F32)
            nc.sync.dma_start(out=rcol_sb, in_=r_col[:, :])
            rpf_sb = const.tile([128, NJJ * 4], F32)
            nc.sync.dma_start(out=rpf_sb, in_=r_pf[:, :])
            epf_sb = const.tile([128, NJJ * 4], I32)
            nc.sync.dma_start(out=epf_sb, in_=e_pf[:, :])
            eall_sb = const.tile([128, NT], I32)
            nc.sync.dma_start(out=eall_sb, in_=e_all[:, :])
            w_sb = const.tile([128, 1024], F32)
            nc.sync.dma_start(out=w_sb, in_=w_t[:, :])

            # ---- lhsT = [exp_hi | exp_lo | ones] per i-tile, bf16 [p, m, t]
            exp_sb = const.tile([128, NT], F32)
            nc.scalar.activation(exp_sb, rcol_sb, mybir.ActivationFunctionType.Exp)
            lhsT = const.tile([128, 3, NT], BF16)
            nc.vector.tensor_copy(lhsT[:, 0, :], exp_sb)          # hi = bf16(exp)
            hi32 = const.tile([128, NT], F32)
            nc.vector.tensor_copy(hi32, lhsT[:, 0, :])            # back to f32
            nc.vector.tensor_sub(lhsT[:, 1, :], exp_sb, hi32)     # lo = bf16(exp-hi)
            nc.vector.memset(lhsT[:, 2, :], 1.0)

            # ---- global e_sum and ||W||^2 reductions (per-partition parts)
            vec3 = const.tile([128, 3], F32)
            e_f = const.tile([128, NT], F32)
            nc.vector.tensor_copy(e_f, eall_sb)                   # i32 -> f32
            nc.vector.tensor_reduce(
                out=vec3[:, 0:1], in_=e_f, axis=mybir.AxisListType.X,
                op=mybir.AluOpType.add,
            )
            w2d = const.tile([128, 1024], F32)
            nc.scalar.activation(
                w2d, w_sb, mybir.ActivationFunctionType.Square,
                accum_out=vec3[:, 1:2],
            )
            epf_f = const.tile([128, NJJ * 4], F32)
            nc.vector.tensor_copy(epf_f, epf_sb)

            # ---- main loop: mask tiles + matmul accumulation
            accs = [psacc.tile([3, 512], F32, tag="acc", name=f"acc{jj}")
                    for jj in range(NJJ)]
            for t in range(NT):
                m = masks.tile([128, JSHARD], BF16)
                if t % ACT_EVERY == ACT_EVERY - 1:
                    s = acttmp.tile([128, JSHARD], F32)
                    nc.scalar.activation(
                        s, yb, Sign, bias=ycol_sb[:, t:t + 1], scale=-1.0,
                    )
                    nc.scalar.activation(m, s, Sign, bias=1.0)
                else:
                    nc.vector.tensor_scalar(
                        out=m, in0=yb, scalar1=ycol_sb[:, t:t + 1], scalar2=None,
                        op0=mybir.AluOpType.is_le,
                    )
                for jj in range(NJJ):
                    nc.tensor.matmul(
                        accs[jj][:, :], lhsT[:, :, t], m[:, 512 * jj:512 * (jj + 1)],
                        start=(t == 0), stop=(t == NT - 1),
                    )

            # ---- per-group epilogue: psum -> row -> pf-layout scatter
            hi_pf = const.tile([128, NJJ * 4], F32)
            lo_pf = const.tile([128, NJJ * 4], F32)
            den_pf = const.tile([128, NJJ * 4], F32)
            for jj in range(NJJ):
                nd = rows.tile([3, 512], F32)
                nc.scalar.copy(nd, accs[jj][:, :])
                nc.sync.dma_start(out=hi_pf[:, 4 * jj:4 * jj + 4], in_=nd[0:1, :])
                nc.sync.dma_start(out=lo_pf[:, 4 * jj:4 * jj + 4], in_=nd[1:2, :])
                nc.sync.dma_start(out=den_pf[:, 4 * jj:4 * jj + 4], in_=nd[2:3, :])

            # ---- wide final math on [128, 16]
            num_pf = const.tile([128, NJJ * 4], F32)
            nc.vector.tensor_add(num_pf, hi_pf, lo_pf)
            lnn = const.tile([128, NJJ * 4], F32)
            nc.scalar.activation(lnn, num_pf, mybir.ActivationFunctionType.Ln)
            lnd = const.tile([128, NJJ * 4], F32)
            nc.scalar.activation(lnd, den_pf, mybir.ActivationFunctionType.Ln)
            s1 = const.tile([128, NJJ * 4], F32)
            nc.vector.tensor_sub(s1, rpf_sb, lnn)
            s2 = const.tile([128, NJJ * 4], F32)
            nc.vector.tensor_add(s2, s1, lnd)
            s3 = const.tile([128, NJJ * 4], F32)
            nc.vector.tensor_mul(s3, s2, epf_f)
            nc.vector.tensor_reduce(
                out=vec3[:, 2:3], in_=s3, axis=mybir.AxisListType.X,
                op=mybir.AluOpType.add,
            )

            # ---- cross-partition fold: [e_sum, w_ssq, t_sum] into one row
            ones_col = const.tile([128, 1], F32)
            nc.vector.memset(ones_col, 1.0)
            sums = pssum.tile([1, 3], F32)
            nc.tensor.matmul(sums[:, :], ones_col, vec3[:, :], start=True, stop=True)

            # ---- assemble out_c = -t_sum / e_sum + (0.01/8) * sqrt(w_ssq)
            inv_e = const.tile([1, 1], F32)
            nc.vector.reciprocal(inv_e, sums[0:1, 0:1])
            lnw = const.tile([1, 1], F32)
            nc.scalar.activation(lnw, sums[0:1, 1:2], mybir.ActivationFunctionType.Ln)
            f1 = const.tile([1, 1], F32)
            # 0.00125 * sqrt(w_ssq) = exp(0.5 * ln(w_ssq) + ln(0.00125))
            lbias = const.tile([1, 1], F32)
            nc.vector.memset(lbias, math.log(0.01 / NCORES))
            nc.scalar.activation(
                f1, lnw, mybir.ActivationFunctionType.Exp,
                scale=0.5, bias=lbias,
            )
            tsc = const.tile([1, 1], F32)
            nc.vector.tensor_mul(tsc, sums[0:1, 2:3], inv_e)
            res = const.tile([1, 1], F32)
            nc.vector.tensor_sub(res, f1, tsc)
            nc.sync.dma_start(out=out[:, :], in_=res)

    return nc


_nc_cache = None


def _get_nc():
    global _nc_cache
    if _nc_cache is None:
        _install_bir_fix()
        _nc_cache = build_kernel()
    return _nc_cache


def make_in_maps(risk_pred, y, e):
    """Host-side sharding: slice/reshape the full inputs for each core."""
    yf = np.ascontiguousarray(y.reshape(NT, 128).T)          # y_col[p,t]=y[t*128+p]
    rf = np.ascontiguousarray(risk_pred.reshape(NT, 128).T)
    ef = np.ascontiguousarray(e.reshape(NT, 128).T)

    in_maps = []
    for c in range(NCORES):
        j0 = c * JSHARD
        ysh = y.reshape(-1)[j0:j0 + JSHARD]
        rsh = risk_pred.reshape(-1)[j0:j0 + JSHARD]
        esh = e.reshape(-1)[j0:j0 + JSHARD]
        # pf layout: x_pf[p, 4*jj + t] = x[j0 + 512*jj + 4*p + t]
        r_pf = np.ascontiguousarray(
            rsh.reshape(NJJ, 128, 4).transpose(1, 0, 2).reshape(128, NJJ * 4))
        e_pf = np.ascontiguousarray(
            esh.reshape(NJJ, 128, 4).transpose(1, 0, 2).reshape(128, NJJ * 4))
        in_maps.append(dict(
            y_col=yf, r_col=rf, e_all=ef,
            y_row=np.ascontiguousarray(ysh.reshape(1, JSHARD)),
            r_pf=r_pf, e_pf=e_pf,
        ))
    return in_maps


def kernel(risk_pred, y, e, W, **run_kwargs):
    nc = _get_nc()
    in_maps = make_in_maps(
        np.asarray(risk_pred, np.float32),
        np.asarray(y, np.float32),
        np.asarray(e, np.int32),
    )
    w_flat = np.ascontiguousarray(np.asarray(W, np.float32).reshape(128, 1024))
    for m in in_maps:
        m["w_t"] = w_flat
    result = run_bass_kernel_spmd(nc, in_maps, core_ids=list(range(NCORES)),
                                  **run_kwargs)
    total = np.float32(0.0)
    for r in result.results:
        total = np.float32(total + r["out"][0, 0])
    kernel.last_result = result
    return np.asarray(total, np.float32)


# revision 6
# speedup vs baseline: 1.0187x; 1.0187x over previous
"""Cox partial-likelihood NegativeLogLikelihood loss on 8 Trainium2 cores.

reference:
    mask[i, j] = (y[j] <= y[i])                       # (N, N)
    num[j] = sum_i exp(r_i) * mask[i, j]
    den[j] = sum_i mask[i, j]
    loss = -sum_j e_j * (r_j - log(num_j / den_j)) / sum_j e_j + 0.01 * ||W||_F

Strategy: shard columns j across the 8 cores (each core owns 2048 columns and
reads the full y / risk_pred, which are tiny).  Per core, the N x 2048 mask is
generated on-chip in [128, 2048] tiles (DVE tensor_scalar is_le, with a share
of tiles on ACT via the double-Sign identity sign(sign(y_i - y_j) + 1)) and
immediately contracted on the TensorEngine against lhsT = [exp_hi, exp_lo, 1]
(bf16, exp = hi + lo Dekker split for f32-grade accuracy) accumulating into
PSUM.  Each core reduces its own columns to a single partial scalar
out_c = -t_c / e_sum + 0.01 * ||W||_F / 8  (e_sum, ||W|| computed redundantly
from the replicated inputs), so the host-side unshard is a pure 8-way sum.
"""
import math

import numpy as np
import orjson

import concourse.bass as bass
import concourse.tile as tile
import concourse.mybir as mybir
from concourse.bass_utils import run_bass_kernel_spmd

F32 = mybir.dt.float32
BF16 = mybir.dt.bfloat16
I32 = mybir.dt.int32

N = 16384
NCORES = 8
JSHARD = N // NCORES            # 2048 columns per core
NT = N // 128                   # 128 i-tiles of 128 rows
NJJ = JSHARD // 512             # 4 matmul column groups per core
N_ACT_TILES = 30                # mask tiles produced on ScalarE (rest on DVE)
ACT_TILES = {round((k + 0.5) * NT / N_ACT_TILES) for k in range(N_ACT_TILES)}

# ---------------------------------------------------------------------------
# Workaround for the installed walrus accepting at most ONE sync-wait command
# per TPB instruction: split multi-wait instructions into preceding
# single-wait EventSemaphore instructions on the same engine.
# ---------------------------------------------------------------------------

def _fix_bir_multiwait(bir_json: bytes) -> bytes:
    d = orjson.loads(bir_json)
    counter = 0
    for fn in d.get("functions", []):
        stack = list(fn.get("blocks", []))
        while stack:
            block = stack.pop()
            stack.extend(block.get("blocks", []))
            new_insts = []
            for inst in block.get("instructions", []):
                sync = inst.get("sync_info") or {}
                waits = sync.get("on_wait") or []
                if len(waits) > 1:
                    for w in waits[:-1]:
                        counter += 1
                        new_insts.append({
                            "debug": inst.get("debug", 0),
                            "engine": inst.get("engine"),
                            "ins": [],
                            "name": f"esw_fix_{counter}",
                            "opcode": "EventSemaphore",
                            "outs": [],
                            "sync_info": {"on_update": [], "on_wait": [w]},
                        })
                    sync["on_wait"] = [waits[-1]]
                new_insts.append(inst)
            block["instructions"] = new_insts
    return orjson.dumps(d)


_patched = False


def _install_bir_fix():
    global _patched
    if _patched:
        return
    _patched = True
    import concourse.bass_utils as bu
    import concourse.bass2jax as b2j

    orig = bu.compile_bir_kernel

    def patched(bir_json, tmpdir, neff_name="file.neff"):
        if isinstance(bir_json, str):
            bir_json = bir_json.encode()
        return orig(_fix_bir_multiwait(bir_json), tmpdir, neff_name)

    bu.compile_bir_kernel = patched
    b2j.compile_bir_kernel = patched


# ---------------------------------------------------------------------------
# Kernel build
# ---------------------------------------------------------------------------

def build_kernel() -> bass.Bass:
    nc = bass.Bass()
    Sign = mybir.ActivationFunctionType.Sign

    y_col = nc.dram_tensor("y_col", [128, NT], F32, kind="ExternalInput")
    r_col = nc.dram_tensor("r_col", [128, NT], F32, kind="ExternalInput")
    y_row = nc.dram_tensor("y_row", [1, JSHARD], F32, kind="ExternalInput")
    r_pf = nc.dram_tensor("r_pf", [128, NJJ * 4], F32, kind="ExternalInput")
    e_pf = nc.dram_tensor("e_pf", [128, NJJ * 4], I32, kind="ExternalInput")
    e_all = nc.dram_tensor("e_all", [128, NT], I32, kind="ExternalInput")
    w_t = nc.dram_tensor("w_t", [128, 1024], F32, kind="ExternalInput")
    out = nc.dram_tensor("out", [1, 1], F32, kind="ExternalOutput")

    with tile.TileContext(nc) as tc:
        with (
            tc.tile_pool(name="const", bufs=1) as const,
            tc.tile_pool(name="masks", bufs=8) as masks,
            tc.tile_pool(name="acttmp", bufs=3) as acttmp,
            tc.tile_pool(name="rows", bufs=2) as rows,
            tc.tile_pool(name="psacc", bufs=4, space="PSUM") as psacc,
            tc.tile_pool(name="pssum", bufs=1, space="PSUM") as pssum,
        ):
            # ---- input loads
            yb = const.tile([128, JSHARD], F32)
            nc.sync.dma_start(out=yb, in_=y_row.ap().to_broadcast([128, JSHARD]))
            ycol_sb = const.tile([128, NT], F32)
            nc.sync.dma_start(out=ycol_sb, in_=y_col[:, :])
            rcol_sb = const.tile([128, NT], F32)
            nc.sync.dma_start(out=rcol_sb, in_=r_col[:, :])
            rpf_sb = const.tile([128, NJJ * 4], F32)
            nc.sync.dma_start(out=rpf_sb, in_=r_pf[:, :])
            epf_sb = const.tile([128, NJJ * 4], I32)
            nc.sync.dma_start(out=epf_sb, in_=e_pf[:, :])
            eall_sb = const.tile([128, NT], I32)
            nc.sync.dma_start(out=eall_sb, in_=e_all[:, :])
            w_sb = const.tile([128, 1024], F32)
            nc.sync.dma_start(out=w_sb, in_=w_t[:, :])

            # ---- lhsT = [exp_hi | exp_lo | ones] per i-tile, bf16 [p, m, t]
            exp_sb = const.tile([128, NT], F32)
            nc.scalar.activation(exp_sb, rcol_sb, mybir.ActivationFunctionType.Exp)
            lhsT = const.tile([128, 3, NT], BF16)
            nc.vector.tensor_copy(lhsT[:, 0, :], exp_sb)          # hi = bf16(exp)
            hi32 = const.tile([128, NT], F32)
            nc.vector.tensor_copy(hi32, lhsT[:, 0, :])            # back to f32
            nc.vector.tensor_sub(lhsT[:, 1, :], exp_sb, hi32)     # lo = bf16(exp-hi)
            nc.vector.memset(lhsT[:, 2, :], 1.0)

            # ---- global e_sum and ||W||^2 reductions (per-partition parts)
            vec3 = const.tile([128, 3], F32)
            e_f = const.tile([128, NT], F32)
            nc.vector.tensor_copy(e_f, eall_sb)                   # i32 -> f32
            nc.vector.tensor_reduce(
                out=vec3[:, 0:1], in_=e_f, axis=mybir.AxisListType.X,
                op=mybir.AluOpType.add,
            )
            w2d = const.tile([128, 1024], F32)
            nc.scalar.activation(
                w2d, w_sb, mybir.ActivationFunctionType.Square,
                accum_out=vec3[:, 1:2],
            )
            epf_f = const.tile([128, NJJ * 4], F32)
            nc.vector.tensor_copy(epf_f, epf_sb)

            # ---- main loop: mask tiles + matmul accumulation
            accs = [psacc.tile([3, 512], F32, tag="acc", name=f"acc{jj}")
                    for jj in range(NJJ)]
            for t in range(NT):
                m = masks.tile([128, JSHARD], BF16)
                if t in ACT_TILES:
                    s = acttmp.tile([128, JSHARD], F32)
                    nc.scalar.activation(
                        s, yb, Sign, bias=ycol_sb[:, t:t + 1], scale=-1.0,
                    )
                    nc.scalar.activation(m, s, Sign, bias=1.0)
                else:
                    nc.vector.tensor_scalar(
                        out=m, in0=yb, scalar1=ycol_sb[:, t:t + 1], scalar2=None,
                        op0=mybir.AluOpType.is_le,
                    )
                for jj in range(NJJ):
                    nc.tensor.matmul(
                        accs[jj][:, :], lhsT[:, :, t], m[:, 512 * jj:512 * (jj + 1)],
                        start=(t == 0), stop=(t == NT - 1),
                    )

            # ---- per-group epilogue: psum -> row -> pf-layout scatter
            hi_pf = const.tile([128, NJJ * 4], F32)
            lo_pf = const.tile([128, NJJ * 4], F32)
            den_pf = const.tile([128, NJJ * 4], F32)
            for jj in range(NJJ):
                nd = rows.tile([3, 512], F32)
                nc.scalar.copy(nd, accs[jj][:, :])
                nc.sync.dma_start(out=hi_pf[:, 4 * jj:4 * jj + 4], in_=nd[0:1, :])
                nc.sync.dma_start(out=lo_pf[:, 4 * jj:4 * jj + 4], in_=nd[1:2, :])
                nc.sync.dma_start(out=den_pf[:, 4 * jj:4 * jj + 4], in_=nd[2:3, :])

            # ---- wide final math on [128, 16]
            num_pf = const.tile([128, NJJ * 4], F32)
            nc.vector.tensor_add(num_pf, hi_pf, lo_pf)
            lnn = const.tile([128, NJJ * 4], F32)
            nc.scalar.activation(lnn, num_pf, mybir.ActivationFunctionType.Ln)
            lnd = const.tile([128, NJJ * 4], F32)
            nc.scalar.activation(lnd, den_pf, mybir.ActivationFunctionType.Ln)
            s1 = const.tile([128, NJJ * 4], F32)
            nc.vector.tensor_sub(s1, rpf_sb, lnn)
            s2 = const.tile([128, NJJ * 4], F32)
            nc.vector.tensor_add(s2, s1, lnd)
            s3 = const.tile([128, NJJ * 4], F32)
            nc.vector.tensor_mul(s3, s2, epf_f)
            nc.vector.tensor_reduce(
                out=vec3[:, 2:3], in_=s3, axis=mybir.AxisListType.X,
                op=mybir.AluOpType.add,
            )

            # ---- cross-partition fold: [e_sum, w_ssq, t_sum] into one row
            ones_col = const.tile([128, 1], F32)
            nc.vector.memset(ones_col, 1.0)
            sums = pssum.tile([1, 3], F32)
            nc.tensor.matmul(sums[:, :], ones_col, vec3[:, :], start=True, stop=True)

            # ---- assemble out_c = -t_sum / e_sum + (0.01/8) * sqrt(w_ssq)
            inv_e = const.tile([1, 1], F32)
            nc.vector.reciprocal(inv_e, sums[0:1, 0:1])
            lnw = const.tile([1, 1], F32)
            nc.scalar.activation(lnw, sums[0:1, 1:2], mybir.ActivationFunctionType.Ln)
            f1 = const.tile([1, 1], F32)
            # 0.00125 * sqrt(w_ssq) = exp(0.5 * ln(w_ssq) + ln(0.00125))
            lbias = const.tile([1, 1], F32)
            nc.vector.memset(lbias, math.log(0.01 / NCORES))
            nc.scalar.activation(
                f1, lnw, mybir.ActivationFunctionType.Exp,
                scale=0.5, bias=lbias,
            )
            tsc = const.tile([1, 1], F32)
            nc.vector.tensor_mul(tsc, sums[0:1, 2:3], inv_e)
            res = const.tile([1, 1], F32)
            nc.vector.tensor_sub(res, f1, tsc)
            nc.sync.dma_start(out=out[:, :], in_=res)

    return nc


_nc_cache = None


def _get_nc():
    global _nc_cache
    if _nc_cache is None:
        _install_bir_fix()
        _nc_cache = build_kernel()
    return _nc_cache


def make_in_maps(risk_pred, y, e):
    """Host-side sharding: slice/reshape the full inputs for each core."""
    yf = np.ascontiguousarray(y.reshape(NT, 128).T)          # y_col[p,t]=y[t*128+p]
    rf = np.ascontiguousarray(risk_pred.reshape(NT, 128).T)
    ef = np.ascontiguousarray(e.reshape(NT, 128).T)

    in_maps = []
    for c in range(NCORES):
        j0 = c * JSHARD
        ysh = y.reshape(-1)[j0:j0 + JSHARD]
        rsh = risk_pred.reshape(-1)[j0:j0 + JSHARD]
        esh = e.reshape(-1)[j0:j0 + JSHARD]
        # pf layout: x_pf[p, 4*jj + t] = x[j0 + 512*jj + 4*p + t]
        r_pf = np.ascontiguousarray(
            rsh.reshape(NJJ, 128, 4).transpose(1, 0, 2).reshape(128, NJJ * 4))
        e_pf = np.ascontiguousarray(
            esh.reshape(NJJ, 128, 4).transpose(1, 0, 2).reshape(128, NJJ * 4))
        in_maps.append(dict(
            y_col=yf, r_col=rf, e_all=ef,
            y_row=np.ascontiguousarray(ysh.reshape(1, JSHARD)),
            r_pf=r_pf, e_pf=e_pf,
        ))
    return in_maps


def kernel(risk_pred, y, e, W, **run_kwargs):
    nc = _get_nc()
    in_maps = make_in_maps(
        np.asarray(risk_pred, np.float32),
        np.asarray(y, np.float32),
        np.asarray(e, np.int32),
    )
    w_flat = np.ascontiguousarray(np.asarray(W, np.float32).reshape(128, 1024))
    for m in in_maps:
        m["w_t"] = w_flat
    result = run_bass_kernel_spmd(nc, in_maps, core_ids=list(range(NCORES)),
                                  **run_kwargs)
    total = np.float32(0.0)
    for r in result.results:
        total = np.float32(total + r["out"][0, 0])
    kernel.last_result = result
    return np.asarray(total, np.float32)


# revision 9
# speedup vs baseline: 1.0309x; 1.0119x over previous
"""Cox partial-likelihood NegativeLogLikelihood loss on 8 Trainium2 cores.

reference:
    mask[i, j] = (y[j] <= y[i])                       # (N, N)
    num[j] = sum_i exp(r_i) * mask[i, j]
    den[j] = sum_i mask[i, j]
    loss = -sum_j e_j * (r_j - log(num_j / den_j)) / sum_j e_j + 0.01 * ||W||_F

Strategy: shard columns j across the 8 cores (each core owns 2048 columns and
reads the full y / risk_pred, which are tiny).  Per core, the N x 2048 mask is
generated on-chip in [128, 2048] tiles (DVE tensor_scalar is_le, with a share
of tiles on ACT via the double-Sign identity sign(sign(y_i - y_j) + 1)) and
immediately contracted on the TensorEngine against lhsT = [exp_hi, exp_lo, 1]
(bf16, exp = hi + lo Dekker split for f32-grade accuracy) accumulating into
PSUM.  Each core reduces its own columns to a single partial scalar
out_c = -t_c / e_sum + 0.01 * ||W||_F / 8  (e_sum, ||W|| computed redundantly
from the replicated inputs), so the host-side unshard is a pure 8-way sum.
"""
import math

import numpy as np
import orjson

import concourse.bass as bass
import concourse.tile as tile
import concourse.mybir as mybir
from concourse.bass_utils import run_bass_kernel_spmd

F32 = mybir.dt.float32
BF16 = mybir.dt.bfloat16
I32 = mybir.dt.int32

N = 16384
NCORES = 8
JSHARD = N // NCORES            # 2048 columns per core
NT = N // 128                   # 128 i-tiles of 128 rows
NJJ = JSHARD // 512             # 4 matmul column groups per core
N_ACT_TILES = 30                # mask tiles produced on ScalarE (rest on DVE)
ACT_TILES = {round((k + 0.5) * NT / N_ACT_TILES) for k in range(N_ACT_TILES)}

# ---------------------------------------------------------------------------
# Workaround for the installed walrus accepting at most ONE sync-wait command
# per TPB instruction: split multi-wait instructions into preceding
# single-wait EventSemaphore instructions on the same engine.
# ---------------------------------------------------------------------------

def _fix_bir_multiwait(bir_json: bytes) -> bytes:
    d = orjson.loads(bir_json)
    counter = 0
    for fn in d.get("functions", []):
        stack = list(fn.get("blocks", []))
        while stack:
            block = stack.pop()
            stack.extend(block.get("blocks", []))
            new_insts = []
            for inst in block.get("instructions", []):
                sync = inst.get("sync_info") or {}
                waits = sync.get("on_wait") or []
                if len(waits) > 1:
                    for w in waits[:-1]:
                        counter += 1
                        new_insts.append({
                            "debug": inst.get("debug", 0),
                            "engine": inst.get("engine"),
                            "ins": [],
                            "name": f"esw_fix_{counter}",
                            "opcode": "EventSemaphore",
                            "outs": [],
                            "sync_info": {"on_update": [], "on_wait": [w]},
                        })
                    sync["on_wait"] = [waits[-1]]
                new_insts.append(inst)
            block["instructions"] = new_insts
    return orjson.dumps(d)


_patched = False


def _install_bir_fix():
    global _patched
    if _patched:
        return
    _patched = True
    import concourse.bass_utils as bu
    import concourse.bass2jax as b2j

    orig = bu.compile_bir_kernel

    def patched(bir_json, tmpdir, neff_name="file.neff"):
        if isinstance(bir_json, str):
            bir_json = bir_json.encode()
        return orig(_fix_bir_multiwait(bir_json), tmpdir, neff_name)

    bu.compile_bir_kernel = patched
    b2j.compile_bir_kernel = patched


# ---------------------------------------------------------------------------
# Kernel build
# ---------------------------------------------------------------------------

def build_kernel() -> bass.Bass:
    nc = bass.Bass()
    Sign = mybir.ActivationFunctionType.Sign

    y_col = nc.dram_tensor("y_col", [128, NT], F32, kind="ExternalInput")
    r_col = nc.dram_tensor("r_col", [128, NT], F32, kind="ExternalInput")
    y_row = nc.dram_tensor("y_row", [1, JSHARD], F32, kind="ExternalInput")
    r_pf = nc.dram_tensor("r_pf", [128, NJJ * 4], F32, kind="ExternalInput")
    e_pf = nc.dram_tensor("e_pf", [128, NJJ * 4], I32, kind="ExternalInput")
    e_all = nc.dram_tensor("e_all", [128, NT], I32, kind="ExternalInput")
    w_t = nc.dram_tensor("w_t", [128, 1024], F32, kind="ExternalInput")
    out = nc.dram_tensor("out", [1, 1], F32, kind="ExternalOutput")

    with tile.TileContext(nc) as tc:
        with (
            tc.tile_pool(name="const", bufs=1) as const,
            tc.tile_pool(name="masks", bufs=8) as masks,
            tc.tile_pool(name="acttmp", bufs=3) as acttmp,
            tc.tile_pool(name="rows", bufs=2) as rows,
            tc.tile_pool(name="psacc", bufs=4, space="PSUM") as psacc,
            tc.tile_pool(name="pssum", bufs=1, space="PSUM") as pssum,
        ):
            # ---- critical-path loads: y broadcast (8 parallel queues) + y_col
            yb = const.tile([128, JSHARD], F32)
            nc.sync.dma_start(out=yb, in_=y_row.ap().to_broadcast([128, JSHARD]))
            ycol_sb = const.tile([128, NT], F32)
            nc.sync.dma_start(out=ycol_sb, in_=y_col[:, :])
            rcol_sb = const.tile([128, NT], F32)
            nc.sync.dma_start(out=rcol_sb, in_=r_col[:, :])

            # ---- lhsT = [exp_hi | exp_lo | ones] per i-tile, bf16 [p, m, t]
            exp_sb = const.tile([128, NT], F32)
            nc.scalar.activation(exp_sb, rcol_sb, mybir.ActivationFunctionType.Exp)
            lhsT = const.tile([128, 3, NT], BF16)
            nc.vector.tensor_copy(lhsT[:, 0, :], exp_sb)          # hi = bf16(exp)
            hi32 = const.tile([128, NT], F32)
            nc.vector.tensor_copy(hi32, lhsT[:, 0, :])            # back to f32
            nc.vector.tensor_sub(lhsT[:, 1, :], exp_sb, hi32)     # lo = bf16(exp-hi)
            nc.vector.memset(lhsT[:, 2, :], 1.0)

            # ---- main loop: mask tiles + matmul accumulation
            accs = [psacc.tile([3, 512], F32, tag="acc", name=f"acc{jj}")
                    for jj in range(NJJ)]
            for t in range(NT):
                m = masks.tile([128, JSHARD], BF16)
                if t in ACT_TILES:
                    s = acttmp.tile([128, JSHARD], F32)
                    nc.scalar.activation(
                        s, yb, Sign, bias=ycol_sb[:, t:t + 1], scale=-1.0,
                    )
                    nc.scalar.activation(m, s, Sign, bias=1.0)
                else:
                    nc.vector.tensor_scalar(
                        out=m, in0=yb, scalar1=ycol_sb[:, t:t + 1], scalar2=None,
                        op0=mybir.AluOpType.is_le,
                    )
                for jj in range(NJJ):
                    nc.tensor.matmul(
                        accs[jj][:, :], lhsT[:, :, t], m[:, 512 * jj:512 * (jj + 1)],
                        start=(t == 0), stop=(t == NT - 1),
                    )

            # ---- non-critical loads + reductions (scheduled during main loop)
            rpf_sb = const.tile([128, NJJ * 4], F32)
            nc.sync.dma_start(out=rpf_sb, in_=r_pf[:, :])
            epf_sb = const.tile([128, NJJ * 4], I32)
            nc.sync.dma_start(out=epf_sb, in_=e_pf[:, :])
            eall_sb = const.tile([128, NT], I32)
            nc.sync.dma_start(out=eall_sb, in_=e_all[:, :])
            w_sb = const.tile([128, 1024], F32)
            nc.sync.dma_start(out=w_sb, in_=w_t[:, :])
            vec3 = const.tile([128, 3], F32)
            e_f = const.tile([128, NT], F32)
            nc.vector.tensor_copy(e_f, eall_sb)                   # i32 -> f32
            nc.vector.tensor_reduce(
                out=vec3[:, 0:1], in_=e_f, axis=mybir.AxisListType.X,
                op=mybir.AluOpType.add,
            )
            w2d = const.tile([128, 1024], F32)
            nc.scalar.activation(
                w2d, w_sb, mybir.ActivationFunctionType.Square,
                accum_out=vec3[:, 1:2],
            )
            epf_f = const.tile([128, NJJ * 4], F32)
            nc.vector.tensor_copy(epf_f, epf_sb)

            # ---- per-group epilogue: psum -> row -> pf-layout scatter
            hi_pf = const.tile([128, NJJ * 4], F32)
            lo_pf = const.tile([128, NJJ * 4], F32)
            den_pf = const.tile([128, NJJ * 4], F32)
            for jj in range(NJJ):
                nd = rows.tile([3, 512], F32, name=f"nd{jj}")
                nc.scalar.copy(nd, accs[jj][:, :])
                nc.sync.dma_start(out=hi_pf[:, 4 * jj:4 * jj + 4], in_=nd[0:1, :])
                nc.sync.dma_start(out=lo_pf[:, 4 * jj:4 * jj + 4], in_=nd[1:2, :])
                nc.sync.dma_start(out=den_pf[:, 4 * jj:4 * jj + 4], in_=nd[2:3, :])

            # ---- wide final math on [128, 16]
            num_pf = const.tile([128, NJJ * 4], F32)
            nc.vector.tensor_add(num_pf, hi_pf, lo_pf)
            lnn = const.tile([128, NJJ * 4], F32)
            nc.scalar.activation(lnn, num_pf, mybir.ActivationFunctionType.Ln)
            lnd = const.tile([128, NJJ * 4], F32)
            nc.scalar.activation(lnd, den_pf, mybir.ActivationFunctionType.Ln)
            s1 = const.tile([128, NJJ * 4], F32)
            nc.vector.tensor_sub(s1, rpf_sb, lnn)
            s2 = const.tile([128, NJJ * 4], F32)
            nc.vector.tensor_add(s2, s1, lnd)
            s3 = const.tile([128, NJJ * 4], F32)
            nc.vector.tensor_mul(s3, s2, epf_f)
            nc.vector.tensor_reduce(
                out=vec3[:, 2:3], in_=s3, axis=mybir.AxisListType.X,
                op=mybir.AluOpType.add,
            )

            # ---- cross-partition fold: [e_sum, w_ssq, t_sum] into one row
            ones_col = const.tile([128, 1], F32)
            nc.vector.memset(ones_col, 1.0)
            sums = pssum.tile([1, 3], F32)
            nc.tensor.matmul(sums[:, :], ones_col, vec3[:, :], start=True, stop=True)

            # ---- assemble out_c = -t_sum / e_sum + (0.01/8) * sqrt(w_ssq)
            inv_e = const.tile([1, 1], F32)
            nc.vector.reciprocal(inv_e, sums[0:1, 0:1])
            lnw = const.tile([1, 1], F32)
            nc.scalar.activation(lnw, sums[0:1, 1:2], mybir.ActivationFunctionType.Ln)
            f1 = const.tile([1, 1], F32)
            # 0.00125 * sqrt(w_ssq) = exp(0.5 * ln(w_ssq) + ln(0.00125))
            lbias = const.tile([1, 1], F32)
            nc.vector.memset(lbias, math.log(0.01 / NCORES))
            nc.scalar.activation(
                f1, lnw, mybir.ActivationFunctionType.Exp,
                scale=0.5, bias=lbias,
            )
            tsc = const.tile([1, 1], F32)
            nc.vector.tensor_mul(tsc, sums[0:1, 2:3], inv_e)
            res = const.tile([1, 1], F32)
            nc.vector.tensor_sub(res, f1, tsc)
            nc.sync.dma_start(out=out[:, :], in_=res)

    return nc


_nc_cache = None


def _get_nc():
    global _nc_cache
    if _nc_cache is None:
        _install_bir_fix()
        _nc_cache = build_kernel()
    return _nc_cache


def make_in_maps(risk_pred, y, e):
    """Host-side sharding: slice/reshape the full inputs for each core."""
    yf = np.ascontiguousarray(y.reshape(NT, 128).T)          # y_col[p,t]=y[t*128+p]
    rf = np.ascontiguousarray(risk_pred.reshape(NT, 128).T)
    ef = np.ascontiguousarray(e.reshape(NT, 128).T)

    in_maps = []
    for c in range(NCORES):
        j0 = c * JSHARD
        ysh = y.reshape(-1)[j0:j0 + JSHARD]
        rsh = risk_pred.reshape(-1)[j0:j0 + JSHARD]
        esh = e.reshape(-1)[j0:j0 + JSHARD]
        # pf layout: x_pf[p, 4*jj + t] = x[j0 + 512*jj + 4*p + t]
        r_pf = np.ascontiguousarray(
            rsh.reshape(NJJ, 128, 4).transpose(1, 0, 2).reshape(128, NJJ * 4))
        e_pf = np.ascontiguousarray(
            esh.reshape(NJJ, 128, 4).transpose(1, 0, 2).reshape(128, NJJ * 4))
        in_maps.append(dict(
            y_col=yf, r_col=rf, e_all=ef,
            y_row=np.ascontiguousarray(ysh.reshape(1, JSHARD)),
            r_pf=r_pf, e_pf=e_pf,
        ))
    return in_maps


def kernel(risk_pred, y, e, W, **run_kwargs):
    nc = _get_nc()
    in_maps = make_in_maps(
        np.asarray(risk_pred, np.float32),
        np.asarray(y, np.float32),
        np.asarray(e, np.int32),
    )
    w_flat = np.ascontiguousarray(np.asarray(W, np.float32).reshape(128, 1024))
    for m in in_maps:
        m["w_t"] = w_flat
    result = run_bass_kernel_spmd(nc, in_maps, core_ids=list(range(NCORES)),
                                  **run_kwargs)
    total = np.float32(0.0)
    for r in result.results:
        total = np.float32(total + r["out"][0, 0])
    kernel.last_result = result
    return np.asarray(total, np.float32)
